# revision 4
# baseline (speedup 1.0000x reference)
"""MiniMoE Trainium2 kernel — expert-parallel, two-tier precision,
paired token blocks sharing weight loads.

Problem (hardcoded): x [4, 2048, 1024] f32, router_w [1024, 4], router_b [4],
w1/w3 [4, 1024, 4096], w2 [4, 4096, 1024], top-2 of 4 experts, SwiGLU.

Strategy
--------
Host computes the (tiny) router + top-2 dispatch. Core pair (2e, 2e+1) owns
expert e: core 2e computes the F in [0, 2048) half of the SwiGLU FFN, core
2e+1 the F in [2048, 4096) half, over all tokens routed to expert e. The
partial outputs sum to the expert output, and the host scatter-adds them
with the renormalized gate weights.

Per token-expert pair there are two tiers, split by gate weight:
  precise: plain bf16 matmuls (384 PE-cycles/token) — ~4e-3 rel err;
  quant: the whole FFN runs in fp8(e4m3) DoubleRow — up-projections on
      fp8 x/w1/w3, h quantized to fp8 by the DVE product (carrying the
      w3 scale 8), down-projection on fp8 w2 (scale 32, removed with
      the 1/256 output copy) — ~6.6e-2 rel err, ~217 cycles/token.
Each expert's tokens are sorted by gate descending; the top Cp go to the
precise tier, the rest (small gates, error contribution ~g^2) to the
quant tier.

Token blocks are processed in PAIRS: the two matmuls that apply one
weight tile to the pair's two moving blocks share a single LDWEIGHTS
(the duplicate load emitted by Bass is removed post-build — verified
bit-identical on HW for bf16 and fp8 DoubleRow). This hides the
256-column DoubleRow weight load (~213 ns, which cannot overlap a
single ~240 ns DR matmul's successor load slot) and halves PE
sequencer load.

fp8 scales (powers of two, folded into existing instructions): x: 1,
w1: 32 (silu input scale 1/32), w3: 8 (h inherits the 8), w2: 32
(output scale 1/256).
"""

import numpy as np
import ml_dtypes

import concourse.bass as bass
import concourse.bacc as bacc
import concourse.tile as tile
import concourse.mybir as mybir
from concourse.bass_utils import run_bass_kernel_spmd

B, S, D, F, E, TOPK = 4, 2048, 1024, 4096, 4, 2
N_CORES = 8
FH = F // 2          # F-half handled per core
P = 128              # SBUF partitions
ND = D // P          # 8 d-blocks (4 DoubleRow pairs)
NF = FH // P         # 16 f-blocks per core
NQ = NF // 2         # 8 f-pair blocks for the DoubleRow down-projection
FP8 = mybir.dt.float8e4
BF16 = mybir.dt.bfloat16
F32 = mybir.dt.float32
DR = mybir.MatmulPerfMode.DoubleRow
E4 = ml_dtypes.float8_e4m3

S_W1, S_W3, S_W2 = 32.0, 8.0, 32.0
CS_TARGET = 1024       # quant-tier width (two full 512 sub-blocks)

_NC_CACHE: dict[tuple, object] = {}


def _pair_blocks(C: int, max_w: int = 512) -> list[list[tuple[int, int]]]:
    """Split C tokens into sub-blocks of <=max_w columns, grouped in pairs
    so each weight load serves two matmuls. Block count is forced even;
    widths are ~balanced and 64-aligned except the last."""
    if C <= 0:
        return []
    nb = (C + max_w - 1) // max_w
    if nb % 2:
        nb += 1
    w = (C // nb) // 64 * 64
    if w == 0:
        w = max(C // nb, 1)
    sizes = [w] * (nb - 1) + [C - w * (nb - 1)]
    assert 0 < sizes[-1] <= max_w, (C, sizes)
    blocks, t0 = [], 0
    for tb in sizes:
        blocks.append((t0, tb))
        t0 += tb
    return [blocks[i:i + 2] for i in range(0, nb, 2)]


def _dedup_ldweights(nc) -> int:
    """Remove InstLdweights that reload the weights the PE already holds
    (same access pattern as the immediately preceding load, no sync info,
    not a dependency target). The following matmuls then reuse the loaded
    weights — verified bit-identical on HW for bf16 and fp8 DoubleRow.
    One-pass slice rewrite: list.remove() compares pyo3 objects and is
    O(N^2) on the 100k-instruction repeat builds."""
    referenced: set[str] = set()
    fn = nc.m.functions[0]
    for blk in fn.blocks:
        for inst in blk.instructions:
            referenced.update(inst.sync_dependency_names())
            referenced.update(inst.nosync_dependency_names())
    removed = 0
    for blk in fn.blocks:
        insts = blk.instructions
        last_key = None
        dead: set[int] = set()
        for inst in insts:
            tn = type(inst).__name__
            if tn == "InstLdweights":
                key = f"{inst.ins[0]}|{inst.perf_mode}"
                if (key == last_key and inst.sync_info is None
                        and inst.name not in referenced):
                    dead.add(id(inst))
                else:
                    last_key = key
            elif tn == "InstMatmult":
                pass                      # matmuls keep the loaded weights
            elif getattr(inst, "engine", None) == mybir.EngineType.PE:
                last_key = None           # drains etc. — be conservative
        if dead:
            insts[:] = [i for i in insts if id(i) not in dead]
            removed += len(dead)
    return removed


def _build_nc(Cp: int, Cs: int, repeat: int = 1):
    """SPMD per-core program: Cp precise (bf16) + Cs quant (fp8 DR) tokens."""
    nc = bacc.Bacc("TRN2", target_bir_lowering=False, debug=False,
                   num_devices=N_CORES)
    xpb = nc.dram_tensor("xpb", [D, Cp], BF16, kind="ExternalInput").ap()
    xs8 = (nc.dram_tensor("xs8", [D, Cs], FP8, kind="ExternalInput").ap()
           if Cs else None)
    w1b = nc.dram_tensor("w1b", [NF, P, ND * P], BF16, kind="ExternalInput").ap()
    w3b = nc.dram_tensor("w3b", [NF, P, ND * P], BF16, kind="ExternalInput").ap()
    w2b = nc.dram_tensor("w2b", [NF, P, D], BF16, kind="ExternalInput").ap()
    w18 = nc.dram_tensor("w18", [NF, P, ND * P], FP8, kind="ExternalInput").ap()
    w38 = nc.dram_tensor("w38", [NF, P, ND * P], FP8, kind="ExternalInput").ap()
    w28 = (nc.dram_tensor("w28", [NQ, P, 2, D], FP8, kind="ExternalInput").ap()
           if Cs else None)
    outT = nc.dram_tensor("outT", [D, Cp + Cs], BF16, kind="ExternalOutput").ap()

    with tile.TileContext(nc) as tc:
        with (
            tc.tile_pool(name="wpool", bufs=1) as wpool,
            tc.tile_pool(name="xpool", bufs=3) as xpool,
            tc.tile_pool(name="x8pool", bufs=2) as x8pool,
            tc.tile_pool(name="hpool", bufs=1) as hpool,
            tc.tile_pool(name="tpool", bufs=2) as tpool,
            tc.tile_pool(name="opool", bufs=3) as opool,
            tc.tile_pool(name="ps1", bufs=1, space=bass.MemorySpace.PSUM) as ps1,
            tc.tile_pool(name="ps2", bufs=2, space=bass.MemorySpace.PSUM) as ps2,
        ):
            xpb_r = xpb.rearrange("(n p) c -> p n c", p=P)
            xs8_r = xs8.rearrange("(n p) c -> p n c", p=P) if Cs else None
            pgroups = _pair_blocks(Cp)
            sgroups = _pair_blocks(Cs) if Cs else []

            # Startup ordering: first psum group needs w1b[0] + x group 0.
            w1b_f, w3b_f, w2b_f, w18_f, w38_f, w28_f = [], [], [], [], [], []

            def wtile(dst, src, ft, tag, dt, shape, rearr):
                t = wpool.tile(shape, dt, tag=f"{tag}_{ft}", name="t")
                nc.sync.dma_start(t[:], src[ft].rearrange(rearr, c=P)
                                  if rearr else src[ft])
                dst.append(t)

            def xload(grp, precise):
                tiles = []
                for (t0, TB) in grp:
                    if precise:
                        t = xpool.tile([P, ND, TB], BF16, tag="xpb", name="x")
                        nc.sync.dma_start(t[:], xpb_r[:, :, t0:t0 + TB])
                    else:
                        t = x8pool.tile([P, ND, TB], FP8, tag="xs8", name="x")
                        nc.sync.dma_start(t[:], xs8_r[:, :, t0:t0 + TB])
                    tiles.append(t)
                return tiles

            wtile(w1b_f, w1b, 0, "w1b", BF16, [P, ND, P], "p (n c) -> p n c")
            x0_tiles = xload(pgroups[0], True)
            wtile(w3b_f, w3b, 0, "w3b", BF16, [P, ND, P], "p (n c) -> p n c")
            for ft in range(1, NF):
                wtile(w1b_f, w1b, ft, "w1b", BF16, [P, ND, P], "p (n c) -> p n c")
                wtile(w3b_f, w3b, ft, "w3b", BF16, [P, ND, P], "p (n c) -> p n c")
            for ft in range(NF):
                t = wpool.tile([P, D], BF16, tag=f"w2b_{ft}", name="t")
                nc.sync.dma_start(t[:], w2b[ft])
                w2b_f.append(t)
            for ft in range(NF):
                wtile(w18_f, w18, ft, "w18", FP8, [P, ND, P], "p (n c) -> p n c")
                wtile(w38_f, w38, ft, "w38", FP8, [P, ND, P], "p (n c) -> p n c")
            if Cs:
                for q in range(NQ):
                    t = wpool.tile([P, 2, D], FP8, tag=f"w28_{q}", name="t")
                    nc.sync.dma_start(t[:], w28[q])
                    w28_f.append(t)

            def do_group(grp, cbase, precise, x_tiles):
                ns = len(grp)
                wtot = sum(TB for _, TB in grp)
                offs = [0]
                for _, TB in grp[:-1]:
                    offs.append(offs[-1] + TB)
                hT = hpool.tile([P, NF, wtot], BF16 if precise else FP8,
                                tag="hT", name="hT")
                for ft in range(NF):
                    p1 = [ps1.tile([P, TB], F32, tag=f"p1_{j}", name="p1")
                          for j, (_, TB) in enumerate(grp)]
                    p3 = [ps1.tile([P, TB], F32, tag=f"p3_{j}", name="p3")
                          for j, (_, TB) in enumerate(grp)]
                    if precise:
                        for wt, ps in ((w1b_f[ft], p1), (w3b_f[ft], p3)):
                            for d in range(ND):
                                for j in range(ns):
                                    nc.tensor.matmul(
                                        ps[j][:], wt[:, d, :], x_tiles[j][:, d, :],
                                        start=(d == 0), stop=(d == ND - 1))
                    else:
                        for wt, ps in ((w18_f[ft], p1), (w38_f[ft], p3)):
                            for p in range(ND // 2):
                                for j in range(ns):
                                    nc.tensor.matmul(
                                        ps[j][:], wt[:, 2 * p:2 * p + 2, :],
                                        x_tiles[j][:, 2 * p:2 * p + 2, :],
                                        start=(p == 0), stop=(p == ND // 2 - 1),
                                        perf_mode=DR)
                    for j, (_, TB) in enumerate(grp):
                        sil = tpool.tile([P, TB], F32, tag="sil", name="sil")
                        nc.scalar.activation(
                            sil[:], p1[j][:], mybir.ActivationFunctionType.Silu,
                            scale=(1.0 if precise else 1.0 / S_W1))
                        nc.vector.tensor_mul(
                            hT[:, ft, offs[j]:offs[j] + TB], sil[:], p3[j][:])

                for db in range(ND):
                    po = [ps2.tile([P, TB], F32, tag=f"po_{j}", name="po")
                          for j, (_, TB) in enumerate(grp)]
                    if precise:
                        for ft in range(NF):
                            for j in range(ns):
                                nc.tensor.matmul(
                                    po[j][:], w2b_f[ft][:, db * P:(db + 1) * P],
                                    hT[:, ft, offs[j]:offs[j] + grp[j][1]],
                                    start=(ft == 0), stop=(ft == NF - 1))
                    else:
                        for q in range(NQ):
                            for j in range(ns):
                                nc.tensor.matmul(
                                    po[j][:],
                                    w28_f[q][:, :, db * P:(db + 1) * P],
                                    hT[:, 2 * q:2 * q + 2,
                                       offs[j]:offs[j] + grp[j][1]],
                                    start=(q == 0), stop=(q == NQ - 1),
                                    perf_mode=DR)
                    for j, (t0, TB) in enumerate(grp):
                        ot = opool.tile([P, TB], BF16, tag="ot", name="ot")
                        nc.scalar.mul(ot[:], po[j][:],
                                      1.0 if precise else 1.0 / (S_W3 * S_W2))
                        nc.sync.dma_start(
                            outT[db * P:(db + 1) * P,
                                 cbase + t0:cbase + t0 + TB],
                            ot[:])

            for r in range(repeat):
                for gi, grp in enumerate(pgroups):
                    if r == 0 and gi == 0:
                        x_tiles = x0_tiles
                    else:
                        x_tiles = xload(grp, True)
                    do_group(grp, 0, True, x_tiles)
                for grp in sgroups:
                    do_group(grp, Cp, False, xload(grp, False))

    n = _dedup_ldweights(nc)
    assert n > 0, "ldweights dedup removed nothing — pairing broken?"
    nc.compile()
    return nc


def _route(x, router_w, router_b):
    """Host router: top-2 expert ids + renormalized gates (float64 math)."""
    T = x.shape[0] * x.shape[1]
    xf = x.reshape(T, D).astype(np.float64)
    logits = xf @ router_w.astype(np.float64) + router_b.astype(np.float64)
    # stable sort: ties resolve to the lowest expert id, like jax.lax.top_k
    order = np.argsort(-logits, axis=-1, kind="stable")   # [T, E] descending
    top_i = order[:, :TOPK]                        # [T, 2]
    top_l = np.take_along_axis(logits, top_i, axis=-1)
    top_l -= top_l.max(axis=-1, keepdims=True)
    ex = np.exp(top_l)
    gates = ex / ex.sum(axis=-1, keepdims=True)    # [T, 2] renormalized
    return top_i, gates


def _q8(a, s):
    return np.clip(a * np.float32(s), -240, 240).astype(E4)


def _tile_w(w):  # [D, FH] -> [NF, P, ND*P]; chunk ft == SBUF tile ft
    return np.ascontiguousarray(
        w.reshape(ND, P, NF, P).transpose(2, 1, 0, 3).reshape(NF, P, ND * P))


def prepare(x, router_w, router_b, w1, w3, w2):
    """Route on host, tier tokens, build per-core input maps."""
    T = x.shape[0] * x.shape[1]
    xf = np.ascontiguousarray(x.reshape(T, D), dtype=np.float32)
    top_i, gates = _route(x, router_w, router_b)

    idx_per_e, gate_per_e = [], []
    for e in range(E):
        mask = (top_i == e)
        rows = np.nonzero(mask.any(axis=-1))[0]
        g = np.where(mask[rows, 0], gates[rows, 0], gates[rows, 1])
        order = np.argsort(-g, kind="stable")      # gate descending
        idx_per_e.append(rows[order])
        gate_per_e.append(g[order].astype(np.float32))

    Cmax = max(max(len(r) for r in idx_per_e), 1)
    Cs = min(CS_TARGET, Cmax // 2)
    Cp = Cmax - Cs
    if Cp > min(len(r) for r in idx_per_e):
        Cp = min(len(r) for r in idx_per_e)
        Cs = Cmax - Cp

    xq8 = _q8(xf, 1.0)
    xb = xf.astype(ml_dtypes.bfloat16)

    in_maps = []
    for core in range(N_CORES):
        e, half = core // 2, core % 2
        fs = slice(half * FH, (half + 1) * FH)
        rows = idx_per_e[e]
        prec, sing = rows[:Cp], rows[Cp:]

        def xt(q, rws, C, dt):
            g = np.zeros((C, D), dt)
            g[:len(rws)] = q[rws]
            return np.ascontiguousarray(g.T)

        w1e = w1[e, :, fs].astype(np.float32)
        w3e = w3[e, :, fs].astype(np.float32)
        w2e = w2[e, fs, :].astype(np.float32)
        m = {
            "xpb": xt(xb, prec, Cp, ml_dtypes.bfloat16),
            "w1b": _tile_w(w1e.astype(ml_dtypes.bfloat16)),
            "w3b": _tile_w(w3e.astype(ml_dtypes.bfloat16)),
            "w2b": np.ascontiguousarray(
                w2e.astype(ml_dtypes.bfloat16).reshape(NF, P, D)),
            "w18": _tile_w(_q8(w1e, S_W1)),
            "w38": _tile_w(_q8(w3e, S_W3)),
        }
        if Cs:
            m["xs8"] = xt(xq8, sing, Cs, E4)
            m["w28"] = np.ascontiguousarray(
                _q8(w2e, S_W2).reshape(NQ, 2, P, D).transpose(0, 2, 1, 3))
        in_maps.append(m)
    meta = (T, Cp, idx_per_e, gate_per_e)
    return Cp, Cs, in_maps, meta


def combine(results, meta):
    """Gate-weighted scatter-add of the per-core partial expert outputs."""
    T, Cp, idx_per_e, gate_per_e = meta
    out = np.zeros((T, D), np.float32)
    for e in range(E):
        rows = idx_per_e[e]
        n = len(rows)
        part = (results[2 * e]["outT"].T[:n].astype(np.float32)
                + results[2 * e + 1]["outT"].T[:n].astype(np.float32))
        out[rows] += gate_per_e[e][:, None] * part
    return out.reshape(B, S, D)


def kernel(**inputs):
    x = np.asarray(inputs["x"], np.float32)
    router_w = np.asarray(inputs["router_w"], np.float32)
    router_b = np.asarray(inputs["router_b"], np.float32)
    w1 = np.asarray(inputs["w1"], np.float32)
    w3 = np.asarray(inputs["w3"], np.float32)
    w2 = np.asarray(inputs["w2"], np.float32)

    Cp, Cs, in_maps, meta = prepare(x, router_w, router_b, w1, w3, w2)
    if (Cp, Cs) not in _NC_CACHE:
        _NC_CACHE[(Cp, Cs)] = _build_nc(Cp, Cs)
    nc = _NC_CACHE[(Cp, Cs)]
    res = run_bass_kernel_spmd(nc, in_maps, list(range(N_CORES)))
    return combine(res.results, meta)


# revision 7
# speedup vs baseline: 1.0317x; 1.0317x over previous
"""MiniMoE Trainium2 kernel — expert-parallel, two-tier precision,
paired token blocks sharing weight loads, pre-tiled DMA layouts.

Problem (hardcoded): x [4, 2048, 1024] f32, router_w [1024, 4], router_b [4],
w1/w3 [4, 1024, 4096], w2 [4, 4096, 1024], top-2 of 4 experts, SwiGLU.

Strategy
--------
Host computes the (tiny) router + top-2 dispatch. Core pair (2e, 2e+1) owns
expert e: core 2e computes the F in [0, 2048) half of the SwiGLU FFN, core
2e+1 the F in [2048, 4096) half, over all tokens routed to expert e. The
partial outputs sum to the expert output, and the host scatter-adds them
with the renormalized gate weights.

Per token-expert pair there are two tiers, split by gate weight
(HW-measured ~185 ns/token vs ~67 ns/token per core):
  precise: plain bf16 matmuls — ~4e-3 rel err;
  quant: the whole FFN in fp8(e4m3) DoubleRow — up-projections on fp8
      x/w1/w3, h quantized to fp8 by the DVE product (carrying the w3
      scale 8), down-projection on fp8 w2 (scale 32, removed with the
      1/256 output copy) — ~6.6e-2 rel err.
Each expert's tokens are sorted by gate descending; the top Cp go to the
precise tier, the rest (small gates, error contribution ~g^2) to the
quant tier.

HW lessons folded in:
  * Token blocks run in PAIRS; the two matmuls applying one weight tile
    to the pair's two moving blocks share a single LDWEIGHTS (duplicate
    loads removed post-build; verified bit-identical on HW).
  * In-loop DMA was ~160 us/pass of serial time: x now loads from a
    pre-tiled DRAM layout (contiguous ~6 KB/partition descriptors
    instead of 1 KB strided rows), and the 8 per-db output stores of a
    group merge into one [P, wtot]-tile store per db issued on the
    Activation HWDGE ring (SP ring keeps the loads), halving ring
    pressure and store count 128 -> ~45.
"""

import numpy as np
import ml_dtypes

import concourse.bass as bass
import concourse.bacc as bacc
import concourse.tile as tile
import concourse.mybir as mybir
from concourse.bass_utils import run_bass_kernel_spmd

B, S, D, F, E, TOPK = 4, 2048, 1024, 4096, 4, 2
N_CORES = 8
FH = F // 2          # F-half handled per core
P = 128              # SBUF partitions
ND = D // P          # 8 d-blocks (4 DoubleRow pairs)
NF = FH // P         # 16 f-blocks per core
NQ = NF // 2         # 8 f-pair blocks for the DoubleRow down-projection
FP8 = mybir.dt.float8e4
BF16 = mybir.dt.bfloat16
F32 = mybir.dt.float32
DR = mybir.MatmulPerfMode.DoubleRow
E4 = ml_dtypes.float8_e4m3

S_W1, S_W3, S_W2 = 32.0, 8.0, 32.0
CS_TARGET = 1152       # quant-tier width (model err ~1.8e-2 vs 2e-2 gate)

_NC_CACHE: dict[tuple, object] = {}


def _pair_blocks(C: int, max_w: int = 512) -> list[list[tuple[int, int]]]:
    """Split C tokens into sub-blocks of <=max_w columns (multiples of 16,
    for fp8 DoubleRow stride rules), grouped in pairs so each weight load
    serves two matmuls."""
    if C <= 0:
        return []
    nb = (C + max_w - 1) // max_w
    if nb % 2:
        nb += 1
    w = min(max_w, -(-C // nb // 16) * 16)   # ceil to multiple of 16
    sizes = [w] * (nb - 1) + [C - w * (nb - 1)]
    assert 0 < sizes[-1] <= max_w, (C, sizes)
    blocks, t0 = [], 0
    for tb in sizes:
        blocks.append((t0, tb))
        t0 += tb
    return [blocks[i:i + 2] for i in range(0, nb, 2)]


def _dedup_ldweights(nc) -> int:
    """Remove InstLdweights that reload the weights the PE already holds
    (same access pattern as the immediately preceding load, no sync info,
    not a dependency target). The following matmuls then reuse the loaded
    weights — verified bit-identical on HW for bf16 and fp8 DoubleRow.
    One-pass slice rewrite: list.remove() compares pyo3 objects and is
    O(N^2) on the 100k-instruction repeat builds."""
    referenced: set[str] = set()
    fn = nc.m.functions[0]
    for blk in fn.blocks:
        for inst in blk.instructions:
            referenced.update(inst.sync_dependency_names())
            referenced.update(inst.nosync_dependency_names())
    removed = 0
    for blk in fn.blocks:
        insts = blk.instructions
        last_key = None
        dead: set[int] = set()
        for inst in insts:
            tn = type(inst).__name__
            if tn == "InstLdweights":
                key = f"{inst.ins[0]}|{inst.perf_mode}"
                if (key == last_key and inst.sync_info is None
                        and inst.name not in referenced):
                    dead.add(id(inst))
                else:
                    last_key = key
            elif tn == "InstMatmult":
                pass                      # matmuls keep the loaded weights
            elif getattr(inst, "engine", None) == mybir.EngineType.PE:
                last_key = None           # drains etc. — be conservative
        if dead:
            insts[:] = [i for i in insts if id(i) not in dead]
            removed += len(dead)
    return removed


def _build_nc(Cp: int, Cs: int, repeat: int = 1):
    """SPMD per-core program: Cp precise (bf16) + Cs quant (fp8 DR) tokens."""
    nc = bacc.Bacc("TRN2", target_bir_lowering=False, debug=False,
                   num_devices=N_CORES)
    pgroups = _pair_blocks(Cp)
    sgroups = _pair_blocks(Cs) if Cs else []
    # Pre-tiled x: per sub-block [P, ND*TB] chunks concatenated along axis 1.
    xpb = nc.dram_tensor("xpb", [P, ND * Cp], BF16, kind="ExternalInput").ap()
    xs8 = (nc.dram_tensor("xs8", [P, ND * Cs], FP8, kind="ExternalInput").ap()
           if Cs else None)
    w1b = nc.dram_tensor("w1b", [NF, P, ND * P], BF16, kind="ExternalInput").ap()
    w3b = nc.dram_tensor("w3b", [NF, P, ND * P], BF16, kind="ExternalInput").ap()
    w2b = nc.dram_tensor("w2b", [NF, P, D], BF16, kind="ExternalInput").ap()
    w18 = nc.dram_tensor("w18", [NF, P, ND * P], FP8, kind="ExternalInput").ap()
    w38 = nc.dram_tensor("w38", [NF, P, ND * P], FP8, kind="ExternalInput").ap()
    w28 = (nc.dram_tensor("w28", [NQ, P, 2, D], FP8, kind="ExternalInput").ap()
           if Cs else None)
    # Pre-tiled out: per group g (col base cb, width w): db-th chunk at
    # flat column ND*cb + db*w, width w.
    outT = nc.dram_tensor("outT", [P, ND * (Cp + Cs)], BF16,
                          kind="ExternalOutput").ap()

    with tile.TileContext(nc) as tc:
        with (
            tc.tile_pool(name="wpool", bufs=1) as wpool,
            tc.tile_pool(name="xpool", bufs=2) as xpool,
            tc.tile_pool(name="x8pool", bufs=2) as x8pool,
            tc.tile_pool(name="hpool", bufs=1) as hpool,
            tc.tile_pool(name="tpool", bufs=2) as tpool,
            tc.tile_pool(name="opool", bufs=2) as opool,
            tc.tile_pool(name="ps1", bufs=1, space=bass.MemorySpace.PSUM) as ps1,
            tc.tile_pool(name="ps2", bufs=2, space=bass.MemorySpace.PSUM) as ps2,
        ):
            w1b_f, w3b_f, w2b_f, w18_f, w38_f, w28_f = [], [], [], [], [], []

            def wtile(dst, src, ft, tag, dt, shape, rearr):
                t = wpool.tile(shape, dt, tag=f"{tag}_{ft}", name="t")
                nc.sync.dma_start(t[:], src[ft].rearrange(rearr, c=P)
                                  if rearr else src[ft])
                dst.append(t)

            def xload(grp, precise):
                tiles = []
                for (t0, TB) in grp:
                    if precise:
                        t = xpool.tile([P, ND, TB], BF16, tag="xpb", name="x")
                        src = xpb[:, ND * t0:ND * (t0 + TB)]
                    else:
                        t = x8pool.tile([P, ND, TB], FP8, tag="xs8", name="x")
                        src = xs8[:, ND * t0:ND * (t0 + TB)]
                    nc.sync.dma_start(
                        t[:], src.rearrange("p (n c) -> p n c", c=TB))
                    tiles.append(t)
                return tiles

            wtile(w1b_f, w1b, 0, "w1b", BF16, [P, ND, P], "p (n c) -> p n c")
            x0_tiles = xload(pgroups[0], True)
            wtile(w3b_f, w3b, 0, "w3b", BF16, [P, ND, P], "p (n c) -> p n c")
            for ft in range(1, NF):
                wtile(w1b_f, w1b, ft, "w1b", BF16, [P, ND, P], "p (n c) -> p n c")
                wtile(w3b_f, w3b, ft, "w3b", BF16, [P, ND, P], "p (n c) -> p n c")
            for ft in range(NF):
                t = wpool.tile([P, D], BF16, tag=f"w2b_{ft}", name="t")
                nc.sync.dma_start(t[:], w2b[ft])
                w2b_f.append(t)
            for ft in range(NF):
                wtile(w18_f, w18, ft, "w18", FP8, [P, ND, P], "p (n c) -> p n c")
                wtile(w38_f, w38, ft, "w38", FP8, [P, ND, P], "p (n c) -> p n c")
            if Cs:
                for q in range(NQ):
                    t = wpool.tile([P, 2, D], FP8, tag=f"w28_{q}", name="t")
                    nc.sync.dma_start(t[:], w28[q])
                    w28_f.append(t)

            def do_group(grp, cbase, precise, x_tiles):
                ns = len(grp)
                wtot = sum(TB for _, TB in grp)
                cb = cbase + grp[0][0]           # absolute column base
                offs = [0]
                for _, TB in grp[:-1]:
                    offs.append(offs[-1] + TB)
                hT = hpool.tile([P, NF, wtot], BF16 if precise else FP8,
                                tag="hT", name="hT")
                for ft in range(NF):
                    p1 = [ps1.tile([P, TB], F32, tag=f"p1_{j}", name="p1")
                          for j, (_, TB) in enumerate(grp)]
                    p3 = [ps1.tile([P, TB], F32, tag=f"p3_{j}", name="p3")
                          for j, (_, TB) in enumerate(grp)]
                    if precise:
                        for wt, ps in ((w1b_f[ft], p1), (w3b_f[ft], p3)):
                            for d in range(ND):
                                for j in range(ns):
                                    nc.tensor.matmul(
                                        ps[j][:], wt[:, d, :], x_tiles[j][:, d, :],
                                        start=(d == 0), stop=(d == ND - 1))
                    else:
                        for wt, ps in ((w18_f[ft], p1), (w38_f[ft], p3)):
                            for p in range(ND // 2):
                                for j in range(ns):
                                    nc.tensor.matmul(
                                        ps[j][:], wt[:, 2 * p:2 * p + 2, :],
                                        x_tiles[j][:, 2 * p:2 * p + 2, :],
                                        start=(p == 0), stop=(p == ND // 2 - 1),
                                        perf_mode=DR)
                    for j, (_, TB) in enumerate(grp):
                        sil = tpool.tile([P, TB], F32, tag="sil", name="sil")
                        nc.scalar.activation(
                            sil[:], p1[j][:], mybir.ActivationFunctionType.Silu,
                            scale=(1.0 if precise else 1.0 / S_W1))
                        nc.vector.tensor_mul(
                            hT[:, ft, offs[j]:offs[j] + TB], sil[:], p3[j][:])

                for db in range(ND):
                    po = [ps2.tile([P, TB], F32, tag=f"po_{j}", name="po")
                          for j, (_, TB) in enumerate(grp)]
                    if precise:
                        for ft in range(NF):
                            for j in range(ns):
                                nc.tensor.matmul(
                                    po[j][:], w2b_f[ft][:, db * P:(db + 1) * P],
                                    hT[:, ft, offs[j]:offs[j] + grp[j][1]],
                                    start=(ft == 0), stop=(ft == NF - 1))
                    else:
                        for q in range(NQ):
                            for j in range(ns):
                                nc.tensor.matmul(
                                    po[j][:],
                                    w28_f[q][:, :, db * P:(db + 1) * P],
                                    hT[:, 2 * q:2 * q + 2,
                                       offs[j]:offs[j] + grp[j][1]],
                                    start=(q == 0), stop=(q == NQ - 1),
                                    perf_mode=DR)
                    ot = opool.tile([P, wtot], BF16, tag="ot", name="ot")
                    for j, (t0, TB) in enumerate(grp):
                        nc.scalar.mul(ot[:, offs[j]:offs[j] + TB], po[j][:],
                                      1.0 if precise else 1.0 / (S_W3 * S_W2))
                    # one store per (group, db) on the ACT HWDGE ring
                    nc.scalar.dma_start(
                        outT[:, ND * cb + db * wtot:ND * cb + (db + 1) * wtot],
                        ot[:])

            for r in range(repeat):
                for gi, grp in enumerate(pgroups):
                    if r == 0 and gi == 0:
                        x_tiles = x0_tiles
                    else:
                        x_tiles = xload(grp, True)
                    do_group(grp, 0, True, x_tiles)
                for grp in sgroups:
                    do_group(grp, Cp, False, xload(grp, False))

    n = _dedup_ldweights(nc)
    assert n > 0, "ldweights dedup removed nothing — pairing broken?"
    nc.compile()
    return nc


def _route(x, router_w, router_b):
    """Host router: top-2 expert ids + renormalized gates (float64 math)."""
    T = x.shape[0] * x.shape[1]
    xf = x.reshape(T, D).astype(np.float64)
    logits = xf @ router_w.astype(np.float64) + router_b.astype(np.float64)
    # stable sort: ties resolve to the lowest expert id, like jax.lax.top_k
    order = np.argsort(-logits, axis=-1, kind="stable")   # [T, E] descending
    top_i = order[:, :TOPK]                        # [T, 2]
    top_l = np.take_along_axis(logits, top_i, axis=-1)
    top_l -= top_l.max(axis=-1, keepdims=True)
    ex = np.exp(top_l)
    gates = ex / ex.sum(axis=-1, keepdims=True)    # [T, 2] renormalized
    return top_i, gates


def _q8(a, s):
    return np.clip(a * np.float32(s), -240, 240).astype(E4)


def _tile_w(w):  # [D, FH] -> [NF, P, ND*P]; chunk ft == SBUF tile ft
    return np.ascontiguousarray(
        w.reshape(ND, P, NF, P).transpose(2, 1, 0, 3).reshape(NF, P, ND * P))


def _stage_x(xT, groups):
    """[D, C] -> [P, ND*C]: per sub-block [P, ND*TB] contiguous chunks."""
    C = xT.shape[1]
    xr = xT.reshape(ND, P, C)
    chunks = [xr[:, :, t0:t0 + TB].transpose(1, 0, 2).reshape(P, ND * TB)
              for grp in groups for (t0, TB) in grp]
    return np.ascontiguousarray(np.concatenate(chunks, axis=1))


def prepare(x, router_w, router_b, w1, w3, w2):
    """Route on host, tier tokens, build per-core input maps."""
    T = x.shape[0] * x.shape[1]
    xf = np.ascontiguousarray(x.reshape(T, D), dtype=np.float32)
    top_i, gates = _route(x, router_w, router_b)

    idx_per_e, gate_per_e = [], []
    for e in range(E):
        mask = (top_i == e)
        rows = np.nonzero(mask.any(axis=-1))[0]
        g = np.where(mask[rows, 0], gates[rows, 0], gates[rows, 1])
        order = np.argsort(-g, kind="stable")      # gate descending
        idx_per_e.append(rows[order])
        gate_per_e.append(g[order].astype(np.float32))

    Cmax = max(max(len(r) for r in idx_per_e), 1)
    Cs = min(CS_TARGET, Cmax // 2)
    Cp = Cmax - Cs
    if Cp > min(len(r) for r in idx_per_e):
        Cp = min(len(r) for r in idx_per_e)
        Cs = Cmax - Cp
    pgroups = _pair_blocks(Cp)
    sgroups = _pair_blocks(Cs) if Cs else []

    xq8 = _q8(xf, 1.0)
    xb = xf.astype(ml_dtypes.bfloat16)

    in_maps = []
    for core in range(N_CORES):
        e, half = core // 2, core % 2
        fs = slice(half * FH, (half + 1) * FH)
        rows = idx_per_e[e]
        prec, sing = rows[:Cp], rows[Cp:]

        def xt(q, rws, C, dt):
            g = np.zeros((C, D), dt)
            g[:len(rws)] = q[rws]
            return np.ascontiguousarray(g.T)

        w1e = w1[e, :, fs].astype(np.float32)
        w3e = w3[e, :, fs].astype(np.float32)
        w2e = w2[e, fs, :].astype(np.float32)
        m = {
            "xpb": _stage_x(xt(xb, prec, Cp, ml_dtypes.bfloat16), pgroups),
            "w1b": _tile_w(w1e.astype(ml_dtypes.bfloat16)),
            "w3b": _tile_w(w3e.astype(ml_dtypes.bfloat16)),
            "w2b": np.ascontiguousarray(
                w2e.astype(ml_dtypes.bfloat16).reshape(NF, P, D)),
            "w18": _tile_w(_q8(w1e, S_W1)),
            "w38": _tile_w(_q8(w3e, S_W3)),
        }
        if Cs:
            m["xs8"] = _stage_x(xt(xq8, sing, Cs, E4), sgroups)
            m["w28"] = np.ascontiguousarray(
                _q8(w2e, S_W2).reshape(NQ, 2, P, D).transpose(0, 2, 1, 3))
        in_maps.append(m)
    meta = (T, Cp, Cs, idx_per_e, gate_per_e)
    return Cp, Cs, in_maps, meta


def _unstage_out(o, Cp, Cs):
    """[P, ND*(Cp+Cs)] staged -> [D, Cp+Cs]."""
    full = np.empty((D, Cp + Cs), o.dtype)
    groups = [(g, 0) for g in _pair_blocks(Cp)] + \
             [(g, Cp) for g in (_pair_blocks(Cs) if Cs else [])]
    for grp, cbase in groups:
        cb = cbase + grp[0][0]
        w = sum(TB for _, TB in grp)
        for db in range(ND):
            full[db * P:(db + 1) * P, cb:cb + w] = \
                o[:, ND * cb + db * w:ND * cb + (db + 1) * w]
    return full


def combine(results, meta):
    """Gate-weighted scatter-add of the per-core partial expert outputs."""
    T, Cp, Cs, idx_per_e, gate_per_e = meta
    out = np.zeros((T, D), np.float32)
    for e in range(E):
        rows = idx_per_e[e]
        n = len(rows)
        part = (
            _unstage_out(results[2 * e]["outT"], Cp, Cs).T[:n].astype(np.float32)
            + _unstage_out(results[2 * e + 1]["outT"], Cp, Cs).T[:n]
            .astype(np.float32))
        out[rows] += gate_per_e[e][:, None] * part
    return out.reshape(B, S, D)


def kernel(**inputs):
    x = np.asarray(inputs["x"], np.float32)
    router_w = np.asarray(inputs["router_w"], np.float32)
    router_b = np.asarray(inputs["router_b"], np.float32)
    w1 = np.asarray(inputs["w1"], np.float32)
    w3 = np.asarray(inputs["w3"], np.float32)
    w2 = np.asarray(inputs["w2"], np.float32)

    Cp, Cs, in_maps, meta = prepare(x, router_w, router_b, w1, w3, w2)
    if (Cp, Cs) not in _NC_CACHE:
        _NC_CACHE[(Cp, Cs)] = _build_nc(Cp, Cs)
    nc = _NC_CACHE[(Cp, Cs)]
    res = run_bass_kernel_spmd(nc, in_maps, list(range(N_CORES)))
    return combine(res.results, meta)
